# revision 4
# baseline (speedup 1.0000x reference)
"""Grouped-query attention kernel for 8 Trainium2 NeuronCores.

Problem (hardcoded): x [2, 512, 16, 16, 16] f32, Wq/Wk/Wv/Wo [512, 512],
biases [512]. G=4 heads of dim 128, N=4096 tokens. out = x + Wo@attn.

Sharding: one (batch, group) pair per core -> 8 cores, no cross-core
communication. Each core computes its group's Q/K/V projections, the
full 4096x4096 attention for its (b, g), and a partial output
projection Wo[:, g_cols] @ O_g -> [512, 4096]. Host sums the 4 partials
per batch and adds the residual + bo.

Device-side layout (per core):
  - xf (x[b] as [512, 4096]) bf16, 4 chunks of [128, 4096] in SBUF
  - Q, K: [128(gs), 4096] bf16; V^T: [128(keys-chunk), 32*128] bf16
  - per query tile (512 wide): S^T chunk = K_chunk^T Q_tile -> PSUM,
    exp on ScalarE (scale folded in) -> E^T bf16, then accumulate
    O += V^T_chunk^T E^T and denom += ones^T E^T on TensorE.
    Normalize via reciprocal + ones-broadcast matmul, then Wo partial.
"""

import os
import numpy as np
import ml_dtypes

B, C, N, G = 2, 512, 4096, 4
GS = C // G          # 128 head dim
SCALE = GS ** -0.5
QT = 512             # query tile width
NQT = N // QT        # 8 query tiles
NKC = N // 128       # 32 key chunks
NCC = C // 128       # 4 contraction chunks for projections
NMC = C // 128       # 4 output-channel chunks

_compiled_nc = None
LAST_RESULT = None


def _build():
    from contextlib import ExitStack
    import concourse.mybir as mybir
    import concourse.tile as tile
    from concourse import bacc

    dt = mybir.dt
    f32 = dt.float32
    bf16 = dt.bfloat16
    Exp = mybir.ActivationFunctionType.Exp

    nc = bacc.Bacc("TRN2", target_bir_lowering=False, debug=False, num_devices=8)

    xb = nc.dram_tensor("xb", [C, N], bf16, kind="ExternalInput")
    wqT = nc.dram_tensor("wqT", [C, GS], bf16, kind="ExternalInput")
    wkT = nc.dram_tensor("wkT", [C, GS], bf16, kind="ExternalInput")
    wvT = nc.dram_tensor("wvT", [C, GS], bf16, kind="ExternalInput")
    woT = nc.dram_tensor("woT", [GS, C], bf16, kind="ExternalInput")
    bq = nc.dram_tensor("bq", [GS, 1], f32, kind="ExternalInput")
    bk = nc.dram_tensor("bk", [GS, 1], f32, kind="ExternalInput")
    bvb = nc.dram_tensor("bvb", [128, GS], f32, kind="ExternalInput")
    outp = nc.dram_tensor("outp", [C, N], f32, kind="ExternalOutput")

    with tile.TileContext(nc) as tc, ExitStack() as ctx:
        persist = ctx.enter_context(tc.tile_pool(name="persist", bufs=1))
        epool = ctx.enter_context(tc.tile_pool(name="epool", bufs=6))
        spool = ctx.enter_context(tc.tile_pool(name="spool", bufs=2))
        psS = ctx.enter_context(tc.tile_pool(name="psS", bufs=2, space="PSUM"))
        psO = ctx.enter_context(tc.tile_pool(name="psO", bufs=2, space="PSUM"))
        psD = ctx.enter_context(tc.tile_pool(name="psD", bufs=2, space="PSUM"))
        psP = ctx.enter_context(tc.tile_pool(name="psP", bufs=2, space="PSUM"))

        def load(shape, dtype, dram_ap, tag):
            t = persist.tile(shape, dtype, tag=tag)
            nc.sync.dma_start(t[:], dram_ap)
            return t

        xf = [load([128, N], bf16, xb[cc * 128:(cc + 1) * 128, :], f"xf{cc}")
              for cc in range(NCC)]
        wq = [load([128, GS], bf16, wqT[cc * 128:(cc + 1) * 128, :], f"wq{cc}")
              for cc in range(NCC)]
        wk = [load([128, GS], bf16, wkT[cc * 128:(cc + 1) * 128, :], f"wk{cc}")
              for cc in range(NCC)]
        wv = [load([128, GS], bf16, wvT[cc * 128:(cc + 1) * 128, :], f"wv{cc}")
              for cc in range(NCC)]
        wo_sb = load([GS, C], bf16, woT[:, :], "wo")
        bq_sb = load([GS, 1], f32, bq[:, :], "bq")
        bk_sb = load([GS, 1], f32, bk[:, :], "bk")
        bvb_sb = load([128, GS], f32, bvb[:, :], "bvb")

        ones_k = persist.tile([128, 1], bf16, tag="ones_k")
        nc.vector.memset(ones_k[:], 1.0)
        ones_1 = persist.tile([1, 128], f32, tag="ones_1")
        nc.vector.memset(ones_1[:], 1.0)

        q_sb = persist.tile([GS, N], bf16, tag="q_sb")
        k_sb = persist.tile([GS, N], bf16, tag="k_sb")
        vt_sb = persist.tile([128, N], bf16, tag="vt_sb")

        # Q / K projections: [gs, N] = W_g @ xf (+ bias per partition)
        for w_t, b_t, dst in ((wq, bq_sb, q_sb), (wk, bk_sb, k_sb)):
            for nt in range(NQT):
                nsl = slice(nt * QT, (nt + 1) * QT)
                ps = psS.tile([128, QT], f32, tag="ps")
                for cc in range(NCC):
                    nc.tensor.matmul(ps[:], w_t[cc][:], xf[cc][:, nsl],
                                     start=(cc == 0), stop=(cc == NCC - 1))
                nc.vector.tensor_scalar_add(dst[:, nsl], ps[:], b_t[:])

        # V^T: [keys, gs] per 128-key chunk = xf_chunk^T @ Wv_g^T (+ bias bcast)
        for kc in range(NKC):
            ksl = slice(kc * 128, (kc + 1) * 128)
            ps = psS.tile([128, QT], f32, tag="ps")
            for cc in range(NCC):
                nc.tensor.matmul(ps[:, :GS], xf[cc][:, ksl], wv[cc][:],
                                 start=(cc == 0), stop=(cc == NCC - 1))
            nc.vector.tensor_add(vt_sb[:, ksl], ps[:, :GS], bvb_sb[:])

        # Attention per query tile
        for qt in range(NQT):
            qsl = slice(qt * QT, (qt + 1) * QT)
            po = psO.tile([128, QT], f32, tag="po")
            pd = psD.tile([1, QT], f32, tag="pd")
            for kc in range(NKC):
                ksl = slice(kc * 128, (kc + 1) * 128)
                ps = psS.tile([128, QT], f32, tag="ps")
                nc.tensor.matmul(ps[:], k_sb[:, ksl], q_sb[:, qsl],
                                 start=True, stop=True)
                e = epool.tile([128, QT], bf16, tag="e")
                nc.scalar.activation(e[:], ps[:], Exp, scale=SCALE)
                nc.tensor.matmul(po[:], vt_sb[:, ksl], e[:],
                                 start=(kc == 0), stop=(kc == NKC - 1))
                nc.tensor.matmul(pd[:], ones_k[:], e[:],
                                 start=(kc == 0), stop=(kc == NKC - 1))
            recip = spool.tile([1, QT], f32, tag="recip")
            nc.vector.reciprocal(recip[:], pd[:])
            pb = psS.tile([128, QT], f32, tag="ps")
            nc.tensor.matmul(pb[:], ones_1[:], recip[:], start=True, stop=True)
            b_sb = spool.tile([128, QT], f32, tag="bsb")
            nc.scalar.copy(b_sb[:], pb[:])
            o_sb = spool.tile([128, QT], bf16, tag="osb")
            nc.vector.tensor_mul(o_sb[:], po[:], b_sb[:])
            for mc in range(NMC):
                msl = slice(mc * 128, (mc + 1) * 128)
                pp = psP.tile([128, QT], f32, tag="pp")
                nc.tensor.matmul(pp[:], wo_sb[:, msl], o_sb[:],
                                 start=True, stop=True)
                st = spool.tile([128, QT], f32, tag="st")
                nc.vector.tensor_copy(st[:], pp[:])
                nc.sync.dma_start(outp[msl, qsl], st[:])

    nc.compile()
    return nc


def _get_compiled():
    global _compiled_nc
    if _compiled_nc is None:
        _compiled_nc = _build()
    return _compiled_nc


def _ensure_ntff_hook():
    """Best-effort: register the axon NTFF profile hook so trace=True
    yields exec_time_ns. The image's antenv lacks axon_hooks; shim it."""
    import sys, types
    try:
        from antenv.axon_hooks import get_axon_ntff_profile_hook  # noqa: F401
        return
    except ImportError:
        pass
    try:
        mod = types.ModuleType("antenv.axon_hooks")
        _hook = [None]
        mod.set_axon_ntff_profile_hook = lambda h: _hook.__setitem__(0, h)
        mod.get_axon_ntff_profile_hook = lambda: _hook[0]
        sys.modules["antenv.axon_hooks"] = mod
        import antenv
        antenv.axon_hooks = mod
        from trn_agent_boot.trn_boot import _ntff_profile_via_ctypes
        mod.set_axon_ntff_profile_hook(
            _ntff_profile_via_ctypes("/opt/axon/libaxon_pjrt.so"))
    except Exception:
        pass


def kernel(x, Wq, bq, Wk, bk, Wv, bv, Wo, bo):
    global LAST_RESULT
    from concourse.bass_utils import run_bass_kernel_spmd

    nc = _get_compiled()
    bf = ml_dtypes.bfloat16
    x = np.asarray(x, dtype=np.float32)
    b, c, d, h, w = x.shape
    n = d * h * w
    xf = x.reshape(b, c, n)
    Wq = np.asarray(Wq, np.float32)
    Wk = np.asarray(Wk, np.float32)
    Wv = np.asarray(Wv, np.float32)
    Wo = np.asarray(Wo, np.float32)
    bq = np.asarray(bq, np.float32)
    bk = np.asarray(bk, np.float32)
    bv = np.asarray(bv, np.float32)
    bo = np.asarray(bo, np.float32)

    in_maps = []
    for core in range(8):
        bb, g = divmod(core, G)
        gsl = slice(g * GS, (g + 1) * GS)
        in_maps.append({
            "xb": np.ascontiguousarray(xf[bb]).astype(bf),
            "wqT": np.ascontiguousarray(Wq[gsl, :].T).astype(bf),
            "wkT": np.ascontiguousarray(Wk[gsl, :].T).astype(bf),
            "wvT": np.ascontiguousarray(Wv[gsl, :].T).astype(bf),
            "woT": np.ascontiguousarray(Wo[:, gsl].T).astype(bf),
            "bq": bq[gsl].reshape(GS, 1).copy(),
            "bk": bk[gsl].reshape(GS, 1).copy(),
            "bvb": np.ascontiguousarray(np.broadcast_to(bv[gsl], (128, GS))),
        })

    trace = bool(os.environ.get("BASS_TRACE"))
    if trace:
        _ensure_ntff_hook()
    LAST_RESULT = run_bass_kernel_spmd(
        nc, in_maps, core_ids=list(range(8)), trace=trace)
    outs = LAST_RESULT.results

    out = np.empty((b, c, n), np.float32)
    for bb in range(b):
        acc = xf[bb] + bo[:, None]
        for g in range(G):
            acc = acc + outs[bb * G + g]["outp"]
        out[bb] = acc
    return out.reshape(b, c, d, h, w)


# revision 6
# speedup vs baseline: 1.1455x; 1.1455x over previous
"""Grouped-query attention kernel for 8 Trainium2 NeuronCores.

Problem (hardcoded): x [2, 512, 16, 16, 16] f32, Wq/Wk/Wv/Wo [512, 512],
biases [512]. G=4 heads of dim 128, N=4096 tokens. out = x + Wo@attn.

Sharding: one (batch, group) pair per core -> 8 cores, no cross-core
communication. Each core computes its group's Q/K/V projections, the
full 4096x4096 attention for its (b, g), and a partial output
projection Wo[:, g_cols] @ O_g -> [512, 4096]. Host sums the 4 partials
per batch and adds the residual + bo.

Device-side layout (per core):
  - xf (x[b] as [512, 4096]) bf16, 4 chunks of [128, 4096] in SBUF
  - Q, K: [128(gs), 4096] bf16; V^T: [128(keys-chunk), 32*128] bf16
  - per query tile (512 wide): S^T chunk = K_chunk^T Q_tile -> PSUM,
    exp on ScalarE (scale folded in) -> E^T bf16, then accumulate
    O += V^T_chunk^T E^T and denom += ones^T E^T on TensorE.
    Normalize via reciprocal + ones-broadcast matmul, then Wo partial.
"""

import os
import numpy as np
import ml_dtypes

B, C, N, G = 2, 512, 4096, 4
GS = C // G          # 128 head dim
SCALE = GS ** -0.5
QT = 512             # query tile width
NQT = N // QT        # 8 query tiles
NKC = N // 128       # 32 key chunks
NCC = C // 128       # 4 contraction chunks for projections
NMC = C // 128       # 4 output-channel chunks

_compiled_nc = None
LAST_RESULT = None


def _build():
    from contextlib import ExitStack
    import concourse.mybir as mybir
    import concourse.tile as tile
    from concourse import bacc

    dt = mybir.dt
    f32 = dt.float32
    bf16 = dt.bfloat16
    Exp = mybir.ActivationFunctionType.Exp

    nc = bacc.Bacc("TRN2", target_bir_lowering=False, debug=False, num_devices=8)

    xb = nc.dram_tensor("xb", [C, N], bf16, kind="ExternalInput")
    wqT = nc.dram_tensor("wqT", [C, GS], bf16, kind="ExternalInput")
    wkT = nc.dram_tensor("wkT", [C, GS], bf16, kind="ExternalInput")
    wvT = nc.dram_tensor("wvT", [C, GS], bf16, kind="ExternalInput")
    woT = nc.dram_tensor("woT", [GS, C], bf16, kind="ExternalInput")
    bq = nc.dram_tensor("bq", [GS, 1], f32, kind="ExternalInput")
    bk = nc.dram_tensor("bk", [GS, 1], f32, kind="ExternalInput")
    bvb = nc.dram_tensor("bvb", [128, GS], f32, kind="ExternalInput")
    outp = nc.dram_tensor("outp", [C, N], f32, kind="ExternalOutput")

    KG = 2                    # key chunks per exp group
    GW = KG * 128             # exp group width in keys
    NGR = N // GW             # 16 groups per query tile

    with tile.TileContext(nc) as tc, ExitStack() as ctx:
        persist = ctx.enter_context(tc.tile_pool(name="persist", bufs=1))
        epool = ctx.enter_context(tc.tile_pool(name="epool", bufs=4))
        spool = ctx.enter_context(tc.tile_pool(name="spool", bufs=2))
        # PSUM budget (8 banks): psS 2x[128,1024]=4, psO 2x[128,512]=2,
        # psD 1, psP 1.
        psS = ctx.enter_context(tc.tile_pool(name="psS", bufs=2, space="PSUM"))
        psO = ctx.enter_context(tc.tile_pool(name="psO", bufs=2, space="PSUM"))
        psD = ctx.enter_context(tc.tile_pool(name="psD", bufs=1, space="PSUM"))
        psP = ctx.enter_context(tc.tile_pool(name="psP", bufs=1, space="PSUM"))

        def load(shape, dtype, dram_ap, tag):
            t = persist.tile(shape, dtype, tag=tag)
            nc.sync.dma_start(t[:], dram_ap)
            return t

        # xf as 4x8 column-block tiles so the first projection matmuls
        # only wait on 4 small DMAs, not the whole 4 MB of x.
        xf = [[load([128, QT], bf16,
                    xb[cc * 128:(cc + 1) * 128, nt * QT:(nt + 1) * QT],
                    f"xf{cc}_{nt}")
               for nt in range(NQT)] for cc in range(NCC)]
        wq = [load([128, GS], bf16, wqT[cc * 128:(cc + 1) * 128, :], f"wq{cc}")
              for cc in range(NCC)]
        wk = [load([128, GS], bf16, wkT[cc * 128:(cc + 1) * 128, :], f"wk{cc}")
              for cc in range(NCC)]
        wv = [load([128, GS], bf16, wvT[cc * 128:(cc + 1) * 128, :], f"wv{cc}")
              for cc in range(NCC)]
        wo_sb = load([GS, C], bf16, woT[:, :], "wo")
        bq_sb = load([GS, 1], f32, bq[:, :], "bq")
        bk_sb = load([GS, 1], f32, bk[:, :], "bk")
        bvb_sb = load([128, GS], f32, bvb[:, :], "bvb")

        ones_k = persist.tile([128, 1], bf16, tag="ones_k")
        nc.vector.memset(ones_k[:], 1.0)
        ones_1 = persist.tile([1, 128], bf16, tag="ones_1")
        nc.vector.memset(ones_1[:], 1.0)

        q_sb = persist.tile([GS, N], bf16, tag="q_sb")
        k_sb = persist.tile([GS, N], bf16, tag="k_sb")
        vt_sb = persist.tile([128, N], bf16, tag="vt_sb")

        # Q / K projections: [gs, N] = W_g @ xf (+ bias per partition)
        for w_t, b_t, dst in ((wq, bq_sb, q_sb), (wk, bk_sb, k_sb)):
            for nt in range(NQT):
                nsl = slice(nt * QT, (nt + 1) * QT)
                ps = psO.tile([128, QT], f32, tag="po")
                for cc in range(NCC):
                    nc.tensor.matmul(ps[:], w_t[cc][:], xf[cc][nt][:],
                                     start=(cc == 0), stop=(cc == NCC - 1))
                nc.vector.tensor_scalar_add(dst[:, nsl], ps[:], b_t[:])

        # V^T: [keys, gs] per 128-key chunk = xf_chunk^T @ Wv_g^T (+ bias bcast)
        for kc in range(NKC):
            ksl = slice(kc * 128, (kc + 1) * 128)
            nt, off = divmod(kc * 128, QT)
            ps = psS.tile([128, GS], f32, tag="ps")
            for cc in range(NCC):
                nc.tensor.matmul(ps[:], xf[cc][nt][:, off:off + 128], wv[cc][:],
                                 start=(cc == 0), stop=(cc == NCC - 1))
            nc.vector.tensor_add(vt_sb[:, ksl], ps[:], bvb_sb[:])

        # Attention, software-pipelined per query tile.
        # PE order per group g: S(g+1) matmuls, then O/D(g) — so PE stays
        # dense while ScalarE runs exp(g). Wide exp over KG key chunks.
        def emit_S(qt, g):
            qsl = slice(qt * QT, (qt + 1) * QT)
            ps = psS.tile([128, GW // 128 * QT], f32, tag="ps")
            for j in range(KG):
                kc = g * KG + j
                ksl = slice(kc * 128, (kc + 1) * 128)
                nc.tensor.matmul(ps[:, j * QT:(j + 1) * QT],
                                 k_sb[:, ksl], q_sb[:, qsl],
                                 start=True, stop=True)
            return ps

        tails = []

        def emit_tail(qt, po, pd):
            def tail():
                qsl = slice(qt * QT, (qt + 1) * QT)
                den_sb = spool.tile([1, QT], bf16, tag="den")
                nc.vector.tensor_copy(den_sb[:], pd[:])
                pb = psP.tile([128, QT], f32, tag="pp")
                nc.tensor.matmul(pb[:], ones_1[:], den_sb[:],
                                 start=True, stop=True)
                binv = spool.tile([128, QT], f32, tag="binv")
                nc.vector.reciprocal(binv[:], pb[:])
                o_sb = spool.tile([128, QT], bf16, tag="osb")
                nc.vector.tensor_mul(o_sb[:], po[:], binv[:])
                for mc in range(NMC):
                    msl = slice(mc * 128, (mc + 1) * 128)
                    pp = psP.tile([128, QT], f32, tag="pp")
                    nc.tensor.matmul(pp[:], wo_sb[:, msl], o_sb[:],
                                     start=True, stop=True)
                    st = spool.tile([128, QT], f32, tag="st")
                    nc.vector.tensor_copy(st[:], pp[:])
                    nc.sync.dma_start(outp[msl, qsl], st[:])
            return tail

        for qt in range(NQT):
            po = psO.tile([128, QT], f32, tag="po")
            pd = psD.tile([1, QT], f32, tag="pd")
            s_cur = emit_S(qt, 0)
            if tails:
                tails.pop()()          # previous q-tile epilogue
            for g in range(NGR):
                s_next = emit_S(qt, g + 1) if g + 1 < NGR else None
                e = epool.tile([128, GW // 128 * QT], bf16, tag="e")
                nc.scalar.activation(e[:], s_cur[:], Exp, scale=SCALE)
                for j in range(KG):
                    kc = g * KG + j
                    ksl = slice(kc * 128, (kc + 1) * 128)
                    esl = slice(j * QT, (j + 1) * QT)
                    nc.tensor.matmul(po[:], vt_sb[:, ksl], e[:, esl],
                                     start=(kc == 0), stop=(kc == NKC - 1))
                    nc.tensor.matmul(pd[:], ones_k[:], e[:, esl],
                                     start=(kc == 0), stop=(kc == NKC - 1))
                s_cur = s_next
            tails.append(emit_tail(qt, po, pd))
        tails.pop()()

    nc.compile()
    return nc


def _get_compiled():
    global _compiled_nc
    if _compiled_nc is None:
        _compiled_nc = _build()
    return _compiled_nc


def _ensure_ntff_hook():
    """Best-effort: register the axon NTFF profile hook so trace=True
    yields exec_time_ns. The image's antenv lacks axon_hooks; shim it."""
    import sys, types
    try:
        from antenv.axon_hooks import get_axon_ntff_profile_hook  # noqa: F401
        return
    except ImportError:
        pass
    try:
        mod = types.ModuleType("antenv.axon_hooks")
        _hook = [None]
        mod.set_axon_ntff_profile_hook = lambda h: _hook.__setitem__(0, h)
        mod.get_axon_ntff_profile_hook = lambda: _hook[0]
        sys.modules["antenv.axon_hooks"] = mod
        import antenv
        antenv.axon_hooks = mod
        from trn_agent_boot.trn_boot import _ntff_profile_via_ctypes
        mod.set_axon_ntff_profile_hook(
            _ntff_profile_via_ctypes("/opt/axon/libaxon_pjrt.so"))
    except Exception:
        pass


def kernel(x, Wq, bq, Wk, bk, Wv, bv, Wo, bo):
    global LAST_RESULT
    from concourse.bass_utils import run_bass_kernel_spmd

    nc = _get_compiled()
    bf = ml_dtypes.bfloat16
    x = np.asarray(x, dtype=np.float32)
    b, c, d, h, w = x.shape
    n = d * h * w
    xf = x.reshape(b, c, n)
    Wq = np.asarray(Wq, np.float32)
    Wk = np.asarray(Wk, np.float32)
    Wv = np.asarray(Wv, np.float32)
    Wo = np.asarray(Wo, np.float32)
    bq = np.asarray(bq, np.float32)
    bk = np.asarray(bk, np.float32)
    bv = np.asarray(bv, np.float32)
    bo = np.asarray(bo, np.float32)

    in_maps = []
    for core in range(8):
        bb, g = divmod(core, G)
        gsl = slice(g * GS, (g + 1) * GS)
        in_maps.append({
            "xb": np.ascontiguousarray(xf[bb]).astype(bf),
            "wqT": np.ascontiguousarray(Wq[gsl, :].T).astype(bf),
            "wkT": np.ascontiguousarray(Wk[gsl, :].T).astype(bf),
            "wvT": np.ascontiguousarray(Wv[gsl, :].T).astype(bf),
            "woT": np.ascontiguousarray(Wo[:, gsl].T).astype(bf),
            "bq": bq[gsl].reshape(GS, 1).copy(),
            "bk": bk[gsl].reshape(GS, 1).copy(),
            "bvb": np.ascontiguousarray(np.broadcast_to(bv[gsl], (128, GS))),
        })

    trace = bool(os.environ.get("BASS_TRACE"))
    if trace:
        _ensure_ntff_hook()
    LAST_RESULT = run_bass_kernel_spmd(
        nc, in_maps, core_ids=list(range(8)), trace=trace)
    outs = LAST_RESULT.results

    out = np.empty((b, c, n), np.float32)
    for bb in range(b):
        acc = xf[bb] + bo[:, None]
        for g in range(G):
            acc = acc + outs[bb * G + g]["outp"]
        out[bb] = acc
    return out.reshape(b, c, d, h, w)


# revision 10
# speedup vs baseline: 1.2506x; 1.0918x over previous
"""Grouped-query attention kernel for 8 Trainium2 NeuronCores.

Problem (hardcoded): x [2, 512, 16, 16, 16] f32, Wq/Wk/Wv/Wo [512, 512],
biases [512]. G=4 heads of dim 128, N=4096 tokens. out = x + Wo@attn.

Sharding: one (batch, group) pair per core -> 8 cores, no cross-core
communication. Each core computes its group's Q/K/V projections, the
full 4096x4096 attention for its (b, g), and a partial output
projection Wo[:, g_cols] @ O_g -> [512, 4096]. Host sums the 4 partials
per batch and adds the residual + bo.

Device-side layout (per core):
  - xf (x[b] as [512, 4096]) bf16, 4 chunks of [128, 4096] in SBUF
  - Q, K: [128(gs), 4096] bf16; V^T: [128(keys-chunk), 32*128] bf16
  - per query tile (512 wide): S^T chunk = K_chunk^T Q_tile -> PSUM,
    exp on ScalarE (scale folded in) -> E^T bf16, then accumulate
    O += V^T_chunk^T E^T and denom += ones^T E^T on TensorE.
    Normalize via reciprocal + ones-broadcast matmul, then Wo partial.
"""

import os
import numpy as np
import ml_dtypes

B, C, N, G = 2, 512, 4096, 4
GS = C // G          # 128 head dim
SCALE = GS ** -0.5
QT = 512             # query tile width
NQT = N // QT        # 8 query tiles
NKC = N // 128       # 32 key chunks
NCC = C // 128       # 4 contraction chunks for projections
NMC = C // 128       # 4 output-channel chunks

_compiled_nc = None
LAST_RESULT = None


def _build():
    from contextlib import ExitStack
    import concourse.mybir as mybir
    import concourse.tile as tile
    from concourse import bacc

    dt = mybir.dt
    f32 = dt.float32
    bf16 = dt.bfloat16
    Exp = mybir.ActivationFunctionType.Exp

    nc = bacc.Bacc("TRN2", target_bir_lowering=False, debug=False, num_devices=8)

    xb = nc.dram_tensor("xb", [C, N], bf16, kind="ExternalInput")
    wqT = nc.dram_tensor("wqT", [C, GS], bf16, kind="ExternalInput")
    wkT = nc.dram_tensor("wkT", [C, GS], bf16, kind="ExternalInput")
    wvT = nc.dram_tensor("wvT", [C, GS], bf16, kind="ExternalInput")
    woT = nc.dram_tensor("woT", [GS, C], bf16, kind="ExternalInput")
    bq = nc.dram_tensor("bq", [GS, 1], f32, kind="ExternalInput")
    bk = nc.dram_tensor("bk", [GS, 1], f32, kind="ExternalInput")
    bvb = nc.dram_tensor("bvb", [128, GS], f32, kind="ExternalInput")
    outp = nc.dram_tensor("outp", [C, N], f32, kind="ExternalOutput")

    KG = 2                    # key chunks per exp group
    GW = KG * 128             # exp group width in keys
    NGR = N // GW             # 16 groups per query tile

    with tile.TileContext(nc) as tc, ExitStack() as ctx:
        persist = ctx.enter_context(tc.tile_pool(name="persist", bufs=1))
        epool = ctx.enter_context(tc.tile_pool(name="epool", bufs=4))
        spool = ctx.enter_context(tc.tile_pool(name="spool", bufs=2))
        # PSUM budget (8 banks): psS 2x[128,1024]=4, psO 2x[128,512]=2,
        # psD 1, psP 1.
        psS = ctx.enter_context(tc.tile_pool(name="psS", bufs=2, space="PSUM"))
        psO = ctx.enter_context(tc.tile_pool(name="psO", bufs=2, space="PSUM"))
        psD = ctx.enter_context(tc.tile_pool(name="psD", bufs=1, space="PSUM"))
        psP = ctx.enter_context(tc.tile_pool(name="psP", bufs=1, space="PSUM"))

        def load(shape, dtype, dram_ap, tag):
            t = persist.tile(shape, dtype, tag=tag)
            nc.sync.dma_start(t[:], dram_ap)
            return t

        # Weights first: they gate the first projection matmuls.
        wq = [load([128, GS], bf16, wqT[cc * 128:(cc + 1) * 128, :], f"wq{cc}")
              for cc in range(NCC)]
        wk = [load([128, GS], bf16, wkT[cc * 128:(cc + 1) * 128, :], f"wk{cc}")
              for cc in range(NCC)]
        wv = [load([128, GS], bf16, wvT[cc * 128:(cc + 1) * 128, :], f"wv{cc}")
              for cc in range(NCC)]
        wo_sb = load([GS, C], bf16, woT[:, :], "wo")
        bq_sb = load([GS, 1], f32, bq[:, :], "bq")
        bk_sb = load([GS, 1], f32, bk[:, :], "bk")
        bvb_sb = load([128, GS], f32, bvb[:, :], "bvb")

        # xf as 4x8 column-block tiles so the first projection matmuls
        # only wait on 4 small DMAs, not the whole 4 MB of x.
        xf = [[load([128, QT], bf16,
                    xb[cc * 128:(cc + 1) * 128, nt * QT:(nt + 1) * QT],
                    f"xf{cc}_{nt}")
               for nt in range(NQT)] for cc in range(NCC)]

        ones_k = persist.tile([128, 1], bf16, tag="ones_k")
        nc.vector.memset(ones_k[:], 1.0)
        ones_1 = persist.tile([1, 128], bf16, tag="ones_1")
        nc.vector.memset(ones_1[:], 1.0)

        q_sb = persist.tile([GS, N], bf16, tag="q_sb")
        k_sb = persist.tile([GS, N], bf16, tag="k_sb")
        vt_sb = persist.tile([128, N], bf16, tag="vt_sb")

        # Q / K projections: [gs, N] = W_g @ xf (+ bias per partition)
        for w_t, b_t, dst in ((wq, bq_sb, q_sb), (wk, bk_sb, k_sb)):
            for nt in range(NQT):
                nsl = slice(nt * QT, (nt + 1) * QT)
                ps = psO.tile([128, QT], f32, tag="po")
                for cc in range(NCC):
                    nc.tensor.matmul(ps[:], w_t[cc][:], xf[cc][nt][:],
                                     start=(cc == 0), stop=(cc == NCC - 1))
                nc.vector.tensor_scalar_add(dst[:, nsl], ps[:], b_t[:])

        # V^T: [keys, gs] per 128-key chunk = xf_chunk^T @ Wv_g^T (+ bias bcast)
        for kc in range(NKC):
            ksl = slice(kc * 128, (kc + 1) * 128)
            nt, off = divmod(kc * 128, QT)
            ps = psS.tile([128, GS], f32, tag="ps")
            for cc in range(NCC):
                nc.tensor.matmul(ps[:], xf[cc][nt][:, off:off + 128], wv[cc][:],
                                 start=(cc == 0), stop=(cc == NCC - 1))
            nc.vector.tensor_add(vt_sb[:, ksl], ps[:], bvb_sb[:])

        # Attention, software-pipelined per query tile.
        # PE order per group g: S(g+1) matmuls, then O/D(g) — so PE stays
        # dense while ScalarE runs exp(g). Wide exp over KG key chunks.
        def emit_S(qt, g):
            qsl = slice(qt * QT, (qt + 1) * QT)
            ps = psS.tile([128, GW // 128 * QT], f32, tag="ps")
            for j in range(KG):
                kc = g * KG + j
                ksl = slice(kc * 128, (kc + 1) * 128)
                nc.tensor.matmul(ps[:, j * QT:(j + 1) * QT],
                                 k_sb[:, ksl], q_sb[:, qsl],
                                 start=True, stop=True)
            return ps

        tails = []

        def emit_tail(qt, po, pd):
            def tail():
                qsl = slice(qt * QT, (qt + 1) * QT)
                den_sb = spool.tile([1, QT], bf16, tag="den")
                nc.vector.tensor_copy(den_sb[:], pd[:])
                pb = psP.tile([128, QT], f32, tag="pp")
                nc.tensor.matmul(pb[:], ones_1[:], den_sb[:],
                                 start=True, stop=True)
                binv = spool.tile([128, QT], f32, tag="binv")
                nc.vector.reciprocal(binv[:], pb[:])
                o_sb = spool.tile([128, QT], bf16, tag="osb")
                nc.vector.tensor_mul(o_sb[:], po[:], binv[:])
                for mc in range(NMC):
                    msl = slice(mc * 128, (mc + 1) * 128)
                    pp = psP.tile([128, QT], f32, tag="pp")
                    nc.tensor.matmul(pp[:], wo_sb[:, msl], o_sb[:],
                                     start=True, stop=True)
                    st = spool.tile([128, QT], f32, tag="st")
                    nc.vector.tensor_copy(st[:], pp[:])
                    nc.sync.dma_start(outp[msl, qsl], st[:])
            return tail

        for qt in range(NQT):
            po = psO.tile([128, QT], f32, tag="po")
            pd = psD.tile([1, QT], f32, tag="pd")
            s_cur = emit_S(qt, 0)
            if tails:
                tails.pop()()          # previous q-tile epilogue
            for g in range(NGR):
                s_next = emit_S(qt, g + 1) if g + 1 < NGR else None
                e = epool.tile([128, GW // 128 * QT], bf16, tag="e")
                nc.scalar.activation(e[:], s_cur[:], Exp, scale=SCALE)
                # same-psum-bank matmuls back-to-back: [O,O] then [D,D]
                # (interleaving accumulating matmuls across banks measured
                # ~1.5x slower per matmul)
                for j in range(KG):
                    kc = g * KG + j
                    ksl = slice(kc * 128, (kc + 1) * 128)
                    esl = slice(j * QT, (j + 1) * QT)
                    nc.tensor.matmul(po[:], vt_sb[:, ksl], e[:, esl],
                                     start=(kc == 0), stop=(kc == NKC - 1))
                for j in range(KG):
                    kc = g * KG + j
                    esl = slice(j * QT, (j + 1) * QT)
                    nc.tensor.matmul(pd[:], ones_k[:], e[:, esl],
                                     start=(kc == 0), stop=(kc == NKC - 1))
                s_cur = s_next
            tails.append(emit_tail(qt, po, pd))
        tails.pop()()

    nc.compile()
    return nc


def _get_compiled():
    global _compiled_nc
    if _compiled_nc is None:
        _compiled_nc = _build()
    return _compiled_nc


def _ensure_ntff_hook():
    """Best-effort: register the axon NTFF profile hook so trace=True
    yields exec_time_ns. The image's antenv lacks axon_hooks; shim it."""
    import sys, types
    try:
        from antenv.axon_hooks import get_axon_ntff_profile_hook  # noqa: F401
        return
    except ImportError:
        pass
    try:
        mod = types.ModuleType("antenv.axon_hooks")
        _hook = [None]
        mod.set_axon_ntff_profile_hook = lambda h: _hook.__setitem__(0, h)
        mod.get_axon_ntff_profile_hook = lambda: _hook[0]
        sys.modules["antenv.axon_hooks"] = mod
        import antenv
        antenv.axon_hooks = mod
        from trn_agent_boot.trn_boot import _ntff_profile_via_ctypes
        mod.set_axon_ntff_profile_hook(
            _ntff_profile_via_ctypes("/opt/axon/libaxon_pjrt.so"))
    except Exception:
        pass


def kernel(x, Wq, bq, Wk, bk, Wv, bv, Wo, bo):
    global LAST_RESULT
    from concourse.bass_utils import run_bass_kernel_spmd

    nc = _get_compiled()
    bf = ml_dtypes.bfloat16
    x = np.asarray(x, dtype=np.float32)
    b, c, d, h, w = x.shape
    n = d * h * w
    xf = x.reshape(b, c, n)
    Wq = np.asarray(Wq, np.float32)
    Wk = np.asarray(Wk, np.float32)
    Wv = np.asarray(Wv, np.float32)
    Wo = np.asarray(Wo, np.float32)
    bq = np.asarray(bq, np.float32)
    bk = np.asarray(bk, np.float32)
    bv = np.asarray(bv, np.float32)
    bo = np.asarray(bo, np.float32)

    in_maps = []
    for core in range(8):
        bb, g = divmod(core, G)
        gsl = slice(g * GS, (g + 1) * GS)
        in_maps.append({
            "xb": np.ascontiguousarray(xf[bb]).astype(bf),
            "wqT": np.ascontiguousarray(Wq[gsl, :].T).astype(bf),
            "wkT": np.ascontiguousarray(Wk[gsl, :].T).astype(bf),
            "wvT": np.ascontiguousarray(Wv[gsl, :].T).astype(bf),
            "woT": np.ascontiguousarray(Wo[:, gsl].T).astype(bf),
            "bq": bq[gsl].reshape(GS, 1).copy(),
            "bk": bk[gsl].reshape(GS, 1).copy(),
            "bvb": np.ascontiguousarray(np.broadcast_to(bv[gsl], (128, GS))),
        })

    trace = bool(os.environ.get("BASS_TRACE"))
    if trace:
        _ensure_ntff_hook()
    LAST_RESULT = run_bass_kernel_spmd(
        nc, in_maps, core_ids=list(range(8)), trace=trace)
    outs = LAST_RESULT.results

    out = np.empty((b, c, n), np.float32)
    for bb in range(b):
        acc = xf[bb] + bo[:, None]
        for g in range(G):
            acc = acc + outs[bb * G + g]["outp"]
        out[bb] = acc
    return out.reshape(b, c, d, h, w)


# revision 14
# speedup vs baseline: 1.2829x; 1.0259x over previous
"""Grouped-query attention kernel for 8 Trainium2 NeuronCores.

Problem (hardcoded): x [2, 512, 16, 16, 16] f32, Wq/Wk/Wv/Wo [512, 512],
biases [512]. G=4 heads of dim 128, N=4096 tokens. out = x + Wo@attn.

Sharding: one (batch, group) pair per core -> 8 cores, no cross-core
communication. Each core computes its group's Q/K/V projections, the
full 4096x4096 attention for its (b, g), and a partial output
projection Wo[:, g_cols] @ O_g -> [512, 4096]. Host sums the 4 partials
per batch and adds the residual + bo.

Device-side layout (per core):
  - xf (x[b] as [512, 4096]) bf16, 4 chunks of [128, 4096] in SBUF
  - Q, K: [128(gs), 4096] bf16; V^T: [128(keys-chunk), 32*128] bf16
  - per query tile (512 wide): S^T chunk = K_chunk^T Q_tile -> PSUM,
    exp on ScalarE (scale folded in) -> E^T bf16, then accumulate
    O += V^T_chunk^T E^T and denom += ones^T E^T on TensorE.
    Normalize via reciprocal + ones-broadcast matmul, then Wo partial.
"""

import os
import numpy as np
import ml_dtypes

B, C, N, G = 2, 512, 4096, 4
GS = C // G          # 128 head dim
SCALE = GS ** -0.5
QT = 512             # query tile width
NQT = N // QT        # 8 query tiles
NKC = N // 128       # 32 key chunks
NCC = C // 128       # 4 contraction chunks for projections
NMC = C // 128       # 4 output-channel chunks

_compiled_nc = None
LAST_RESULT = None


def _build():
    from contextlib import ExitStack
    import concourse.mybir as mybir
    import concourse.tile as tile
    from concourse import bacc

    dt = mybir.dt
    f32 = dt.float32
    bf16 = dt.bfloat16
    Exp = mybir.ActivationFunctionType.Exp

    nc = bacc.Bacc("TRN2", target_bir_lowering=False, debug=False, num_devices=8)

    xb = nc.dram_tensor("xb", [C, N], bf16, kind="ExternalInput")
    wqT = nc.dram_tensor("wqT", [C, GS], bf16, kind="ExternalInput")
    wkT = nc.dram_tensor("wkT", [C, GS], bf16, kind="ExternalInput")
    wvT = nc.dram_tensor("wvT", [C, GS], bf16, kind="ExternalInput")
    woT = nc.dram_tensor("woT", [GS, C], bf16, kind="ExternalInput")
    bq = nc.dram_tensor("bq", [GS, 1], f32, kind="ExternalInput")
    bk = nc.dram_tensor("bk", [GS, 1], f32, kind="ExternalInput")
    bvb = nc.dram_tensor("bvb", [128, GS], f32, kind="ExternalInput")
    outp = nc.dram_tensor("outp", [C, N], f32, kind="ExternalOutput")

    KG = 2                    # key chunks per exp group
    GW = KG * 128             # exp group width in keys
    NGR = N // GW             # 16 groups per query tile

    with tile.TileContext(nc) as tc, ExitStack() as ctx:
        persist = ctx.enter_context(tc.tile_pool(name="persist", bufs=1))
        epool = ctx.enter_context(tc.tile_pool(name="epool", bufs=4))
        spool = ctx.enter_context(tc.tile_pool(name="spool", bufs=2))
        # PSUM budget (8 banks): psS 2x[128,1024]=4, psO 2x[128,512]=2,
        # psD 1, psP 1.
        psS = ctx.enter_context(tc.tile_pool(name="psS", bufs=2, space="PSUM"))
        psO = ctx.enter_context(tc.tile_pool(name="psO", bufs=2, space="PSUM"))
        psD = ctx.enter_context(tc.tile_pool(name="psD", bufs=1, space="PSUM"))
        psP = ctx.enter_context(tc.tile_pool(name="psP", bufs=1, space="PSUM"))

        def load(shape, dtype, dram_ap, tag):
            t = persist.tile(shape, dtype, tag=tag)
            nc.sync.dma_start(t[:], dram_ap)
            return t

        # Weights first: they gate the first projection matmuls.
        wq = [load([128, GS], bf16, wqT[cc * 128:(cc + 1) * 128, :], f"wq{cc}")
              for cc in range(NCC)]
        wk = [load([128, GS], bf16, wkT[cc * 128:(cc + 1) * 128, :], f"wk{cc}")
              for cc in range(NCC)]
        wv = [load([128, GS], bf16, wvT[cc * 128:(cc + 1) * 128, :], f"wv{cc}")
              for cc in range(NCC)]
        wo_sb = load([GS, C], bf16, woT[:, :], "wo")
        bq_sb = load([GS, 1], f32, bq[:, :], "bq")
        bk_sb = load([GS, 1], f32, bk[:, :], "bk")
        bvb_sb = load([128, GS], f32, bvb[:, :], "bvb")

        # xf as 4x8 column-block tiles, loaded nt-major so the first
        # projection matmuls (which need all 4 cc chunks of nt=0) are
        # gated by the first 4 DMAs, not the first 25.
        xf = [[None] * NQT for _ in range(NCC)]
        for nt in range(NQT):
            for cc in range(NCC):
                xf[cc][nt] = load(
                    [128, QT], bf16,
                    xb[cc * 128:(cc + 1) * 128, nt * QT:(nt + 1) * QT],
                    f"xf{cc}_{nt}")

        ones_k = persist.tile([128, 1], bf16, tag="ones_k")
        nc.vector.memset(ones_k[:], 1.0)
        ones_1 = persist.tile([1, 128], bf16, tag="ones_1")
        nc.vector.memset(ones_1[:], 1.0)

        q_sb = persist.tile([GS, N], bf16, tag="q_sb")
        k_sb = persist.tile([GS, N], bf16, tag="k_sb")
        vt_sb = persist.tile([128, N], bf16, tag="vt_sb")

        # Q / K projections: [gs, N] = W_g @ xf (+ bias per partition)
        for w_t, b_t, dst in ((wq, bq_sb, q_sb), (wk, bk_sb, k_sb)):
            for nt in range(NQT):
                nsl = slice(nt * QT, (nt + 1) * QT)
                ps = psO.tile([128, QT], f32, tag="po")
                for cc in range(NCC):
                    nc.tensor.matmul(ps[:], w_t[cc][:], xf[cc][nt][:],
                                     start=(cc == 0), stop=(cc == NCC - 1))
                nc.vector.tensor_scalar_add(dst[:, nsl], ps[:], b_t[:])

        # V^T: [keys, gs] per 128-key chunk = xf_chunk^T @ Wv_g^T (+ bias bcast)
        for kc in range(NKC):
            ksl = slice(kc * 128, (kc + 1) * 128)
            nt, off = divmod(kc * 128, QT)
            ps = psS.tile([128, GS], f32, tag="ps")
            for cc in range(NCC):
                nc.tensor.matmul(ps[:], xf[cc][nt][:, off:off + 128], wv[cc][:],
                                 start=(cc == 0), stop=(cc == NCC - 1))
            nc.vector.tensor_add(vt_sb[:, ksl], ps[:], bvb_sb[:])

        # Attention, software-pipelined per query tile.
        # PE order per group g: S(g+1) matmuls, then O/D(g) — so PE stays
        # dense while ScalarE runs exp(g). Wide exp over KG key chunks.
        def emit_S(qt, g):
            qsl = slice(qt * QT, (qt + 1) * QT)
            ps = psS.tile([128, GW // 128 * QT], f32, tag="ps")
            for j in range(KG):
                kc = g * KG + j
                ksl = slice(kc * 128, (kc + 1) * 128)
                nc.tensor.matmul(ps[:, j * QT:(j + 1) * QT],
                                 k_sb[:, ksl], q_sb[:, qsl],
                                 start=True, stop=True)
            return ps

        tails = []

        def emit_tail(qt, po, pd):
            state = {}

            def tail_pre():
                # free the pd bank + start the reciprocal chain early
                den_sb = spool.tile([1, QT], bf16, tag="den")
                nc.vector.tensor_copy(den_sb[:], pd[:])
                pb = psP.tile([128, QT], f32, tag="pp")
                nc.tensor.matmul(pb[:], ones_1[:], den_sb[:],
                                 start=True, stop=True)
                binv = spool.tile([128, QT], f32, tag="binv")
                nc.vector.reciprocal(binv[:], pb[:])
                state["binv"] = binv

            def tail_main():
                qsl = slice(qt * QT, (qt + 1) * QT)
                o_sb = spool.tile([128, QT], bf16, tag="osb")
                nc.vector.tensor_mul(o_sb[:], po[:], state["binv"][:])
                for mc in range(NMC):
                    msl = slice(mc * 128, (mc + 1) * 128)
                    pp = psP.tile([128, QT], f32, tag="pp")
                    nc.tensor.matmul(pp[:], wo_sb[:, msl], o_sb[:],
                                     start=True, stop=True)
                    st = spool.tile([128, QT], f32, tag="st")
                    nc.vector.tensor_copy(st[:], pp[:])
                    nc.sync.dma_start(outp[msl, qsl], st[:])
            return tail_pre, tail_main

        for qt in range(NQT):
            po = psO.tile([128, QT], f32, tag="po")
            s_cur = emit_S(qt, 0)
            if tails:
                tails[-1][0]()         # prev epilogue: den copy + bcast + recip
            pd = psD.tile([1, QT], f32, tag="pd")
            for g in range(NGR):
                # rest of prev epilogue two groups in: by now its DVE
                # chain is done, so the Wo matmuls don't stall PE
                if g == 2 and tails:
                    tails.pop()[1]()
                s_next = emit_S(qt, g + 1) if g + 1 < NGR else None
                e = epool.tile([128, GW // 128 * QT], bf16, tag="e")
                nc.scalar.activation(e[:], s_cur[:], Exp, scale=SCALE)
                # same-psum-bank matmuls back-to-back: [O,O] then [D,D]
                # (interleaving accumulating matmuls across banks measured
                # ~1.5x slower per matmul)
                for j in range(KG):
                    kc = g * KG + j
                    ksl = slice(kc * 128, (kc + 1) * 128)
                    esl = slice(j * QT, (j + 1) * QT)
                    nc.tensor.matmul(po[:], vt_sb[:, ksl], e[:, esl],
                                     start=(kc == 0), stop=(kc == NKC - 1))
                for j in range(KG):
                    kc = g * KG + j
                    esl = slice(j * QT, (j + 1) * QT)
                    nc.tensor.matmul(pd[:], ones_k[:], e[:, esl],
                                     start=(kc == 0), stop=(kc == NKC - 1))
                s_cur = s_next
            tails.append(emit_tail(qt, po, pd))
        tp, tm = tails.pop()
        tp()
        tm()

    nc.compile()
    return nc


def _get_compiled():
    global _compiled_nc
    if _compiled_nc is None:
        _compiled_nc = _build()
    return _compiled_nc


def _ensure_ntff_hook():
    """Best-effort: register the axon NTFF profile hook so trace=True
    yields exec_time_ns. The image's antenv lacks axon_hooks; shim it."""
    import sys, types
    try:
        from antenv.axon_hooks import get_axon_ntff_profile_hook  # noqa: F401
        return
    except ImportError:
        pass
    try:
        mod = types.ModuleType("antenv.axon_hooks")
        _hook = [None]
        mod.set_axon_ntff_profile_hook = lambda h: _hook.__setitem__(0, h)
        mod.get_axon_ntff_profile_hook = lambda: _hook[0]
        sys.modules["antenv.axon_hooks"] = mod
        import antenv
        antenv.axon_hooks = mod
        from trn_agent_boot.trn_boot import _ntff_profile_via_ctypes
        mod.set_axon_ntff_profile_hook(
            _ntff_profile_via_ctypes("/opt/axon/libaxon_pjrt.so"))
    except Exception:
        pass


def kernel(x, Wq, bq, Wk, bk, Wv, bv, Wo, bo):
    global LAST_RESULT
    from concourse.bass_utils import run_bass_kernel_spmd

    nc = _get_compiled()
    bf = ml_dtypes.bfloat16
    x = np.asarray(x, dtype=np.float32)
    b, c, d, h, w = x.shape
    n = d * h * w
    xf = x.reshape(b, c, n)
    Wq = np.asarray(Wq, np.float32)
    Wk = np.asarray(Wk, np.float32)
    Wv = np.asarray(Wv, np.float32)
    Wo = np.asarray(Wo, np.float32)
    bq = np.asarray(bq, np.float32)
    bk = np.asarray(bk, np.float32)
    bv = np.asarray(bv, np.float32)
    bo = np.asarray(bo, np.float32)

    in_maps = []
    for core in range(8):
        bb, g = divmod(core, G)
        gsl = slice(g * GS, (g + 1) * GS)
        in_maps.append({
            "xb": np.ascontiguousarray(xf[bb]).astype(bf),
            "wqT": np.ascontiguousarray(Wq[gsl, :].T).astype(bf),
            "wkT": np.ascontiguousarray(Wk[gsl, :].T).astype(bf),
            "wvT": np.ascontiguousarray(Wv[gsl, :].T).astype(bf),
            "woT": np.ascontiguousarray(Wo[:, gsl].T).astype(bf),
            "bq": bq[gsl].reshape(GS, 1).copy(),
            "bk": bk[gsl].reshape(GS, 1).copy(),
            "bvb": np.ascontiguousarray(np.broadcast_to(bv[gsl], (128, GS))),
        })

    trace = bool(os.environ.get("BASS_TRACE"))
    if trace:
        _ensure_ntff_hook()
    LAST_RESULT = run_bass_kernel_spmd(
        nc, in_maps, core_ids=list(range(8)), trace=trace)
    outs = LAST_RESULT.results

    out = np.empty((b, c, n), np.float32)
    for bb in range(b):
        acc = xf[bb] + bo[:, None]
        for g in range(G):
            acc = acc + outs[bb * G + g]["outp"]
        out[bb] = acc
    return out.reshape(b, c, d, h, w)
